# revision 19
# baseline (speedup 1.0000x reference)
"""EpisodicMemory retrieval kernel for 8 Trainium2 NeuronCores (v4).

Sharding (hardcoded for the nn_EpisodicMemory problem):
  - q = buffer_states.reshape(-1) [25600]: contraction-sharded for layer 1
    (core i gets q[3200i:3200(i+1)] and W1 rows [3200i:3200(i+1), :]),
    partial pre-activations summed with an on-device AllReduce (the only
    collective).
  - W2/W3 replicated; every core computes the full enc locally.
  - episodes_encoded row-sharded: core i scores episodes [1250i:1250(i+1))
    against enc entirely on the TensorEngine: the host supplies the shard
    TRANSPOSED ([128, 32, 1250] fp8), dots = enc-column x epT matmuls and
    norms = ones x square(epT) matmuls, both accumulating in flat [1,1250]
    PSUM (no layout bounces). Local top-8 via max8/max_index, exact fp32
    rescore of the 8 (indirect gather, 128KB), decode all 8 with bf16
    Wd1/Wd2.
  - host merges the 8x8 candidates: global top-3 by exact sims, then
    means the matching decoded vectors (pure gather/selection glue).

All bulk tensors are pre-swizzled on the host into their exact SBUF
layout ([128 partitions, ...] C-order) so each stream is a handful of
big contiguous DMAs (16-engine, ~25KB per partition line) instead of
thousands of 1KB descriptors.

Precision (validated in fp64 numpy against this dataset): candidate
generation runs fully in fp8 (weights pre-scaled by 64 to clear the fp8
subnormal range; enc scale is ranking-invariant), candidate selection
uses exact fp32 rows vs fp32 enc (margin 1.3e-3 vs noise <1e-4), decode
uses fp32 rows + bf16 weights (~4e-3 output rel err).
"""

import numpy as np

DIM = 256
WIN = 100
COMP = 16
NEP = 10000
NCORES = 8

Q = WIN * DIM            # 25600
H1 = 4 * DIM             # 1024
H2 = 2 * DIM             # 512
E = COMP * DIM           # 4096
QS = Q // NCORES         # 3200 rows of W1 per core
ES = NEP // NCORES       # 1250 episodes per core
K = 3
NC = 8                   # candidates per core (top-8 window)
EPS = 1e-5
W_SCALE = 64.0           # host multiplies W1/W2/W3 by this before fp8 cast

_compiled = {}


def build_kernel(gelu_func_name: str = "Gelu", zero_bias=False, unit_affine=False):
    import concourse.bacc as bacc
    import concourse.bass as bass
    import concourse.tile as tile
    import concourse.mybir as mybir

    f32 = mybir.dt.float32
    u32 = mybir.dt.uint32
    bf16 = mybir.dt.bfloat16
    fp8 = mybir.dt.float8e4
    AF = mybir.ActivationFunctionType
    GELU = getattr(AF, gelu_func_name)
    OP = mybir.AluOpType
    DS = 1.0 / W_SCALE

    nc = bacc.Bacc("TRN2", target_bir_lowering=False, debug=False,
                   enable_asserts=True, num_devices=NCORES)

    C1 = H1 // 128   # 8
    C2 = H2 // 128   # 4
    NKC = QS // 128  # 25
    CE = E // 128    # 32
    # episode column chunks for the 1250-wide PE accumulations (<=512 each)
    NSPLIT = [(0, 512), (512, 1024), (1024, ES)]

    # ---- I/O (all bulk tensors pre-swizzled to SBUF layout on host) ----
    q_s = nc.dram_tensor("q_s", [128, NKC], fp8, kind="ExternalInput").ap()
    W1sw = nc.dram_tensor("W1sw", [128, NKC, H1], fp8, kind="ExternalInput").ap()
    W2sw = nc.dram_tensor("W2sw", [128, C1, H2], fp8, kind="ExternalInput").ap()
    W3sw = nc.dram_tensor("W3sw", [128, C2, E], fp8, kind="ExternalInput").ap()
    epT = nc.dram_tensor("epT", [128, CE, ES], fp8, kind="ExternalInput").ap()
    ep32 = nc.dram_tensor("ep32", [ES, E], f32, kind="ExternalInput").ap()
    Wd1sw = nc.dram_tensor("Wd1sw", [128, CE, H2], bf16, kind="ExternalInput").ap()
    Wd2sw = nc.dram_tensor("Wd2sw", [128, C2, DIM], bf16, kind="ExternalInput").ap()
    eye8 = nc.dram_tensor("eye8", [NC, NC], f32, kind="ExternalInput").ap()
    vecs = {}
    if not zero_bias:
        for nm, width in [("b1v", H1), ("b2v", H2), ("b3v", E), ("bd1v", H2),
                          ("bd2v", DIM)]:
            vecs[nm] = nc.dram_tensor(nm, [width], f32, kind="ExternalInput").ap()
    if not unit_affine:
        for nm, width in [("g1v", H1), ("be1v", H1), ("g2v", H2), ("be2v", H2),
                          ("gdv", H2), ("bedv", H2)]:
            vecs[nm] = nc.dram_tensor(nm, [width], f32, kind="ExternalInput").ap()

    loc_out = nc.dram_tensor("loc_out", [NC, DIM], f32, kind="ExternalOutput").ap()
    loc_sims = nc.dram_tensor("loc_sims", [NC], f32, kind="ExternalOutput").ap()

    with tile.TileContext(nc) as tc:
        with tc.tile_pool(name="dram", bufs=1, space="DRAM") as dram, \
             tc.tile_pool(name="const", bufs=1) as const, \
             tc.tile_pool(name="small", bufs=1) as small, \
             tc.tile_pool(name="sqp", bufs=3) as sqp, \
             tc.tile_pool(name="psum", bufs=2, space="PSUM") as psum, \
             tc.tile_pool(name="psum_flat", bufs=1, space="PSUM") as psum_flat, \
             tc.tile_pool(name="psum_tp", bufs=2, space="PSUM") as psum_tp:

            # ---------- bulk streams on the sync ring, priority order ----------
            qsb = const.tile([128, NKC], fp8, tag="qsb")
            nc.sync.dma_start(out=qsb[:, :], in_=q_s[:, :])
            w1sb = const.tile([128, NKC, H1], fp8, tag="w1sb")
            bounds1 = [0, 7, 13, 19, NKC]
            for c in range(4):
                a, b = bounds1[c], bounds1[c + 1]
                nc.sync.dma_start(out=w1sb[:, a:b, :], in_=W1sw[:, a:b, :])

            epsb = const.tile([128, CE, ES], fp8, tag="epsb")
            for c in range(4):
                nc.sync.dma_start(out=epsb[:, 8 * c:8 * (c + 1), :],
                                  in_=epT[:, 8 * c:8 * (c + 1), :])

            w2sb = const.tile([128, C1, H2], fp8, tag="w2sb")
            nc.sync.dma_start(out=w2sb[:, :, :], in_=W2sw[:, :, :])
            w3sb = const.tile([128, C2, E], fp8, tag="w3sb")
            nc.sync.dma_start(out=w3sb[:, :, :], in_=W3sw[:, :, :])
            wd1sb = const.tile([128, CE, H2], bf16, tag="wd1sb")
            for c in range(2):
                nc.sync.dma_start(out=wd1sb[:, 16 * c:16 * (c + 1), :],
                                  in_=Wd1sw[:, 16 * c:16 * (c + 1), :])
            wd2sb = const.tile([128, C2, DIM], bf16, tag="wd2sb")
            nc.sync.dma_start(out=wd2sb[:, :, :], in_=Wd2sw[:, :, :])

            # ---------- small constants on the scalar ring ----------
            eye8sb = const.tile([NC, NC], f32, tag="eye8sb")
            nc.scalar.dma_start(out=eye8sb[:, :], in_=eye8[:, :])
            eye1 = const.tile([1, 1], f32, tag="eye1")
            nc.vector.memset(eye1[:, :], 1.0)
            ones_col = const.tile([128, 1], fp8, tag="ones_col")
            nc.vector.memset(ones_col[:, :], 1.0)
            eps1 = const.tile([1, 1], f32, tag="eps1")
            nc.vector.memset(eps1[:, :], EPS)
            eps8 = const.tile([NC, 1], f32, tag="eps8")
            nc.vector.memset(eps8[:, :], EPS)

            def cvec(nm, width, tag):
                t = const.tile([1, width], f32, tag=tag)
                nc.scalar.dma_start(
                    out=t[:, :], in_=vecs[nm].rearrange("(a n) -> a n", a=1))
                return t

            def cvec_b(nm, width, tag):
                t = const.tile([NC, width], f32, tag=tag)
                nc.scalar.dma_start(
                    out=t[:, :],
                    in_=vecs[nm].rearrange("(a n) -> a n", a=1)
                        .to_broadcast([NC, width]))
                return t

            b1sb = cvec("b1v", H1, "b1sb") if not zero_bias else None
            b2sb = cvec("b2v", H2, "b2sb") if not zero_bias else None
            b3sb = cvec("b3v", E, "b3sb") if not zero_bias else None
            bd1sb = cvec_b("bd1v", H2, "bd1sb") if not zero_bias else None
            bd2sb = cvec_b("bd2v", DIM, "bd2sb") if not zero_bias else None
            g1sb = cvec("g1v", H1, "g1sb") if not unit_affine else None
            be1sb = cvec("be1v", H1, "be1sb") if not unit_affine else None
            g2sb = cvec("g2v", H2, "g2sb") if not unit_affine else None
            be2sb = cvec("be2v", H2, "be2sb") if not unit_affine else None
            gdsb = cvec_b("gdv", H2, "gdsb") if not unit_affine else None
            bedsb = cvec_b("bedv", H2, "bedsb") if not unit_affine else None

            # warm-up collective: pays the ncfw wake-up latency early,
            # overlapped with the weight/episode stream
            warm_in = dram.tile([8], f32)
            warm_out = dram.tile([8], f32)
            warm_sb = small.tile([1, 8], f32, tag="warm")
            nc.vector.memset(warm_sb[:, :], 0.0)
            nc.scalar.dma_start(out=warm_in.rearrange("(a n) -> a n", a=1),
                                in_=warm_sb[:, :])
            nc.gpsimd.collective_compute(
                "AllReduce", OP.add,
                replica_groups=[list(range(NCORES))],
                ins=[warm_in.opt()], outs=[warm_out.opt()],
            )

            # DRAM bounce/scratch
            ar1_in = dram.tile([H1], f32)
            ar1_out = dram.tile([H1], f32)
            enc_dbf = dram.tile([E], bf16)
            enc_df = dram.tile([E], f32)
            idx_d = dram.tile([NC], u32)

            # ======== E1: h1_pre = q_s @ (64*W1) -> psum [1, 1024] ========
            e1pa = psum.tile([1, 512], f32, tag="mm")
            e1pb = psum.tile([1, 512], f32, tag="mm")
            for kc in range(NKC):
                for h, pp in ((0, e1pa), (1, e1pb)):
                    nc.tensor.matmul(
                        out=pp[:, :],
                        lhsT=qsb[:, kc:kc + 1],
                        rhs=w1sb[:, kc, 512 * h:512 * (h + 1)],
                        start=(kc == 0), stop=(kc == NKC - 1),
                    )
            h1f = small.tile([1, H1], f32, tag="h1flat")
            nc.vector.tensor_copy(out=h1f[:, :512], in_=e1pa[:, :])
            nc.vector.tensor_copy(out=h1f[:, 512:], in_=e1pb[:, :])
            nc.scalar.dma_start(out=ar1_in.rearrange("(a n) -> a n", a=1),
                                in_=h1f[:, :])
            nc.gpsimd.collective_compute(
                "AllReduce", OP.add,
                replica_groups=[list(range(NCORES))],
                ins=[ar1_in.opt()], outs=[ar1_out.opt()],
            )

            # ======== episode norms on PE (pre-enc, fills the AR window) ======
            # nsq[n] = sum_k epT[k,n]^2 = ones.T @ square(epT)
            nsq_p = psum_flat.tile([1, ES], f32, tag="flatp")
            for kc in range(CE):
                sq = sqp.tile([128, ES], fp8, tag="sq")
                nc.scalar.activation(out=sq[:, :], in_=epsb[:, kc, :],
                                     func=AF.Square)
                for ci, (a, b) in enumerate(NSPLIT):
                    nc.tensor.matmul(
                        out=nsq_p[:, a:b], lhsT=ones_col[:, :], rhs=sq[:, a:b],
                        start=(kc == 0), stop=(kc == CE - 1),
                    )
            nsqf = small.tile([1, ES], f32, tag="nsqf")
            nc.vector.tensor_copy(out=nsqf[:, :], in_=nsq_p[:, :])
            rstd = small.tile([1, ES], f32, tag="rstdf")
            nc.scalar.activation(out=rstd[:, :], in_=nsqf[:, :], func=AF.Sqrt)
            nc.vector.reciprocal(out=rstd[:, :], in_=rstd[:, :])

            def ln_flat(xf, xout, width, bsb, gsb, besb, name, pre_scale=1.0):
                """LN(gelu(xf*pre_scale + b)) on [1,width] f32 -> xout (f32)."""
                if bsb is not None:
                    if pre_scale != 1.0:
                        nc.scalar.activation(out=xf[:, :], in_=xf[:, :],
                                             func=AF.Copy, scale=pre_scale)
                    nc.vector.tensor_add(out=xf[:, :], in0=xf[:, :], in1=bsb[:, :])
                    nc.scalar.activation(out=xf[:, :], in_=xf[:, :], func=GELU)
                else:
                    nc.scalar.activation(out=xf[:, :], in_=xf[:, :], func=GELU,
                                         scale=pre_scale)
                nsub = (width + 511) // 512
                st = small.tile([1, nsub, 6], f32, tag=f"st_{name}")
                for sg in range(nsub):
                    nc.vector.bn_stats(out=st[:, sg, :],
                                       in_=xf[:, 512 * sg:512 * (sg + 1)])
                mv = small.tile([1, 2], f32, tag=f"mv_{name}")
                nc.vector.bn_aggr(out=mv[:, :], in_=st[:, :, :])
                rs = small.tile([1, 1], f32, tag=f"rstd_{name}")
                nc.scalar.activation(out=rs[:, :], in_=mv[:, 1:2], func=AF.Sqrt,
                                     bias=eps1[:, :])
                nc.vector.reciprocal(out=rs[:, :], in_=rs[:, :])
                last = xout if gsb is None else xf
                nc.vector.tensor_scalar(
                    out=last[:, :], in0=xf[:, :],
                    scalar1=mv[:, 0:1], scalar2=rs[:, :],
                    op0=OP.subtract, op1=OP.mult,
                )
                if gsb is not None:
                    nc.vector.tensor_mul(out=xf[:, :], in0=xf[:, :], in1=gsb[:, :])
                    nc.vector.tensor_add(out=xout[:, :], in0=xf[:, :], in1=besb[:, :])

            def col_pack(src, n_kc, dst):
                """src [1, n_kc*128] f32 -> dst [128, n_kc] (cast to dst dtype)
                via PE transposes (no DRAM bounce)."""
                for kc in range(n_kc):
                    tp = psum_tp.tile([128, NC], f32, tag="tp")
                    nc.tensor.transpose(out=tp[:, :1],
                                        in_=src[:, 128 * kc:128 * (kc + 1)],
                                        identity=eye1[:, :])
                    nc.vector.tensor_copy(out=dst[:, kc:kc + 1], in_=tp[:, :1])

            # ---------- E1 epilogue ----------
            h1 = small.tile([1, H1], f32, tag="h1flat")
            nc.scalar.dma_start(out=h1[:, :],
                                in_=ar1_out.rearrange("(a n) -> a n", a=1))
            h1n = small.tile([1, H1], f32, tag="h1n")
            ln_flat(h1, h1n, H1, b1sb, g1sb, be1sb, "l1", pre_scale=DS)
            h1m = small.tile([128, C1], fp8, tag="h1m")
            col_pack(h1n, C1, h1m)

            # ======== E2 ========
            e23p = psum.tile([1, H2], f32, tag="mm")
            for kc in range(C1):
                nc.tensor.matmul(
                    out=e23p[:, :], lhsT=h1m[:, kc:kc + 1], rhs=w2sb[:, kc, :],
                    start=(kc == 0), stop=(kc == C1 - 1),
                )
            h2 = small.tile([1, H2], f32, tag="h2flat")
            nc.vector.tensor_copy(out=h2[:, :], in_=e23p[:, :])
            h2n = small.tile([1, H2], f32, tag="h2n")
            ln_flat(h2, h2n, H2, b2sb, g2sb, be2sb, "l2", pre_scale=DS)
            h2m = small.tile([128, C2], fp8, tag="h2m")
            col_pack(h2n, C2, h2m)

            # ======== E3: full enc = h2 @ W3 (descale on PSUM copy) ========
            encf = small.tile([1, E], f32, tag="encf")
            for cg in range(4):
                for h in range(2):
                    e3p = psum.tile([1, 512], f32, tag="mm")
                    for kc in range(C2):
                        nc.tensor.matmul(
                            out=e3p[:, :],
                            lhsT=h2m[:, kc:kc + 1],
                            rhs=w3sb[:, kc,
                                     1024 * cg + 512 * h:1024 * cg + 512 * (h + 1)],
                            start=(kc == 0), stop=(kc == C2 - 1),
                        )
                    nc.vector.tensor_scalar_mul(
                        out=encf[:, 1024 * cg + 512 * h:1024 * cg + 512 * (h + 1)],
                        in0=e3p[:, :], scalar1=DS)
            if b3sb is not None:
                nc.vector.tensor_add(out=encf[:, :], in0=encf[:, :], in1=b3sb[:, :])

            # enc -> [128, 32] fp8 columns via DMA-transpose (bf16 bounce)
            encbf = small.tile([1, E], bf16, tag="encbf")
            nc.vector.tensor_copy(out=encbf[:, :], in_=encf[:, :])
            nc.scalar.dma_start(out=enc_dbf.rearrange("(a n) -> a n", a=1),
                                in_=encbf[:, :])
            encm_bf = small.tile([128, CE], bf16, tag="encm_bf")
            nc.sync.dma_start_transpose(
                out=encm_bf[:, :], in_=enc_dbf.rearrange("(kc p) -> kc p", p=128))
            encm = small.tile([128, CE], fp8, tag="encm")
            nc.vector.tensor_copy(out=encm[:, :], in_=encm_bf[:, :])
            # enc broadcast to 8 partitions (f32) for the exact rescore
            nc.scalar.dma_start(out=enc_df.rearrange("(a n) -> a n", a=1),
                                in_=encf[:, :])
            enc8b = small.tile([NC, E], f32, tag="enc8b")
            nc.scalar.dma_start(
                out=enc8b[:, :],
                in_=enc_df.rearrange("(a n) -> a n", a=1).to_broadcast([NC, E]))

            # ======== dots on PE: dot[n] = sum_kc enc_col(kc) . epT[kc][:,n] ====
            dot_p = psum_flat.tile([1, ES], f32, tag="flatp")
            for kc in range(CE):
                for a, b in NSPLIT:
                    nc.tensor.matmul(
                        out=dot_p[:, a:b], lhsT=encm[:, kc:kc + 1],
                        rhs=epsb[:, kc, a:b],
                        start=(kc == 0), stop=(kc == CE - 1),
                    )
            # ======== normalize + local top-8 (read dots from PSUM) ========
            snorm = small.tile([1, ES], f32, tag="snorm")
            nc.vector.tensor_mul(out=snorm[:, :], in0=dot_p[:, :], in1=rstd[:, :])
            vals = small.tile([1, NC], f32, tag="vals")
            nc.vector.max(out=vals[:, :], in_=snorm[:, :])
            idx8 = small.tile([1, NC], u32, tag="idx8")
            nc.vector.max_index(out=idx8[:, :], in_max=vals[:, :],
                                in_values=snorm[:, :])
            nc.scalar.dma_start(out=idx_d.rearrange("(a n) -> a n", a=1),
                                in_=idx8[:, :])
            idxc = small.tile([NC, 1], u32, tag="idxc")
            nc.scalar.dma_start(out=idxc[:, :],
                                in_=idx_d.rearrange("(p o) -> p o", o=1))

            # ======== exact rescore of the 8 candidates ========
            rows8 = small.tile([NC, E], f32, tag="encf")  # reuses encf
            nc.gpsimd.indirect_dma_start(
                out=rows8[:, :], out_offset=None,
                in_=ep32[:, :],
                in_offset=bass.IndirectOffsetOnAxis(ap=idxc[:, :1], axis=0),
            )
            trash8 = small.tile([NC, E], bf16, tag="trash8")
            dots8 = small.tile([NC, 1], f32, tag="dots8")
            nsq8 = small.tile([NC, 1], f32, tag="nsq8")
            nc.vector.tensor_tensor(out=trash8[:, :], in0=rows8[:, :],
                                    in1=enc8b[:, :], op=OP.mult)
            nc.vector.tensor_reduce(out=dots8[:, :], in_=trash8[:, :],
                                    axis=mybir.AxisListType.X, op=OP.add)
            nc.scalar.activation(out=trash8[:, :], in_=rows8[:, :],
                                 func=AF.Square, accum_out=nsq8[:, :])
            nstd8 = small.tile([NC, 1], f32, tag="nstd8")
            nc.scalar.activation(out=nstd8[:, :], in_=nsq8[:, :], func=AF.Sqrt)
            nc.vector.reciprocal(out=nstd8[:, :], in_=nstd8[:, :])
            sim8 = small.tile([NC, 1], f32, tag="sim8")
            nc.vector.tensor_mul(out=sim8[:, :], in0=dots8[:, :], in1=nstd8[:, :])
            nc.scalar.dma_start(out=loc_sims.rearrange("(p o) -> p o", o=1),
                                in_=sim8[:, :])

            # ======== decoder: all 8 candidates ========
            rowsT = small.tile([128, CE, NC], bf16, tag="rowsT")
            pdp = psum.tile([NC, H2], f32, tag="mm")
            for kc in range(CE):
                tp = psum_tp.tile([128, NC], f32, tag="tp")
                nc.tensor.transpose(out=tp[:, :],
                                    in_=rows8[:, 128 * kc:128 * (kc + 1)],
                                    identity=eye8sb[:, :])
                nc.vector.tensor_copy(out=rowsT[:, kc, :], in_=tp[:, :])
            for kc in range(CE):
                nc.tensor.matmul(
                    out=pdp[:, :], lhsT=rowsT[:, kc, :], rhs=wd1sb[:, kc, :],
                    start=(kc == 0), stop=(kc == CE - 1),
                )
            d = small.tile([NC, H2], f32, tag="d")
            nc.vector.tensor_copy(out=d[:, :], in_=pdp[:, :])
            if bd1sb is not None:
                nc.vector.tensor_add(out=d[:, :], in0=d[:, :], in1=bd1sb[:, :])
            nc.scalar.activation(out=d[:, :], in_=d[:, :], func=GELU)
            std = small.tile([NC, 6], f32, tag="std")
            nc.vector.bn_stats(out=std[:, :], in_=d[:, :])
            mvd = small.tile([NC, 2], f32, tag="mvd")
            nc.vector.bn_aggr(out=mvd[:, :], in_=std[:, :])
            rstdd = small.tile([NC, 1], f32, tag="rstdd")
            nc.scalar.activation(out=rstdd[:, :], in_=mvd[:, 1:2], func=AF.Sqrt,
                                 bias=eps8[:, :])
            nc.vector.reciprocal(out=rstdd[:, :], in_=rstdd[:, :])
            nc.vector.tensor_scalar(
                out=d[:, :], in0=d[:, :],
                scalar1=mvd[:, 0:1], scalar2=rstdd[:, :],
                op0=OP.subtract, op1=OP.mult,
            )
            if gdsb is not None:
                nc.vector.tensor_mul(out=d[:, :], in0=d[:, :], in1=gdsb[:, :])
                nc.vector.tensor_add(out=d[:, :], in0=d[:, :], in1=bedsb[:, :])

            dT = small.tile([128, C2, NC], bf16, tag="dT")
            o3p = psum.tile([NC, DIM], f32, tag="mm")
            for kc in range(C2):
                tp = psum_tp.tile([128, NC], f32, tag="tp")
                nc.tensor.transpose(out=tp[:, :],
                                    in_=d[:, 128 * kc:128 * (kc + 1)],
                                    identity=eye8sb[:, :])
                nc.vector.tensor_copy(out=dT[:, kc, :], in_=tp[:, :])
                nc.tensor.matmul(
                    out=o3p[:, :], lhsT=dT[:, kc, :], rhs=wd2sb[:, kc, :],
                    start=(kc == 0), stop=(kc == C2 - 1),
                )
            o3 = small.tile([NC, DIM], f32, tag="o3")
            nc.vector.tensor_copy(out=o3[:, :], in_=o3p[:, :])
            if bd2sb is not None:
                nc.vector.tensor_add(out=o3[:, :], in0=o3[:, :], in1=bd2sb[:, :])

            nc.sync.dma_start(out=loc_out[:, :], in_=o3[:, :])

    nc.compile()
    return nc


def _bf16(a):
    import ml_dtypes
    return np.ascontiguousarray(
        np.asarray(a, dtype=np.float32).astype(ml_dtypes.bfloat16))


def _fp8(a):
    import ml_dtypes
    return np.ascontiguousarray(
        np.asarray(a, dtype=np.float32).astype(ml_dtypes.float8_e4m3))


def _swizzle(w, n_kc):
    """[n_kc*128, n] row-major -> [128, n_kc, n] C-order (SBUF layout)."""
    w = np.asarray(w, dtype=np.float32)
    n = w.shape[1]
    return np.ascontiguousarray(
        w.reshape(n_kc, 128, n).transpose(1, 0, 2))


def _shard_inputs(buffer_states, episodes_encoded, W1, b1, g1, be1, W2, b2, g2,
                  be2, W3, b3, Wd1, bd1, gd, bed, Wd2, bd2, zero_bias,
                  unit_affine):
    q = np.ascontiguousarray(buffer_states, dtype=np.float32).reshape(-1)
    eye8 = np.eye(NC, dtype=np.float32)
    W2c = _fp8(_swizzle(np.asarray(W2, dtype=np.float32) * W_SCALE, H1 // 128))
    W3c = _fp8(_swizzle(np.asarray(W3, dtype=np.float32) * W_SCALE, H2 // 128))
    Wd1c = _bf16(_swizzle(Wd1, E // 128))
    Wd2c = _bf16(_swizzle(Wd2, H2 // 128))
    ep32 = np.ascontiguousarray(episodes_encoded, dtype=np.float32)
    in_maps = []
    for i in range(NCORES):
        qs = q[QS * i:QS * (i + 1)]
        shard = ep32[ES * i:ES * (i + 1)]                     # [1250, 4096]
        # epT [128, 32, 1250]: epT[p, kc, n] = shard[n, 128*kc + p]
        epTc = _fp8(np.ascontiguousarray(
            shard.T.reshape(E // 128, 128, ES).transpose(1, 0, 2)))
        m = {
            "q_s": _fp8(np.ascontiguousarray(qs.reshape(QS // 128, 128).T)),
            "W1sw": _fp8(_swizzle(
                np.asarray(W1[QS * i:QS * (i + 1)], dtype=np.float32) * W_SCALE,
                QS // 128)),
            "W2sw": W2c,
            "W3sw": W3c,
            "epT": epTc,
            "ep32": shard,
            "Wd1sw": Wd1c,
            "Wd2sw": Wd2c,
            "eye8": eye8,
        }
        if not zero_bias:
            m.update({"b1v": b1, "b2v": b2, "b3v": b3, "bd1v": bd1, "bd2v": bd2})
        if not unit_affine:
            m.update({"g1v": g1, "be1v": be1, "g2v": g2, "be2v": be2,
                      "gdv": gd, "bedv": bed})
        in_maps.append(m)
    return in_maps


def _merge(results):
    sims = np.concatenate([r["loc_sims"] for r in results])              # [64]
    outs = np.concatenate([r["loc_out"] for r in results], axis=0)       # [64, 256]
    top = np.argsort(-sims, kind="stable")[:K]
    return outs[top].mean(axis=0).astype(np.float32)


def kernel(*, trace=False, **inputs):
    from concourse.bass_utils import run_bass_kernel_spmd

    k = int(inputs.pop("k"))
    assert k == K, f"kernel hardcodes k=3, got {k}"
    arrs = {name: np.ascontiguousarray(np.asarray(v, dtype=np.float32))
            for name, v in inputs.items()}
    zero_bias = all(not arrs[n].any() for n in ("b1", "b2", "b3", "bd1", "bd2"))
    unit_affine = (all(np.all(arrs[n] == 1.0) for n in ("g1", "g2", "gd")) and
                   all(not arrs[n].any() for n in ("be1", "be2", "bed")))
    in_maps = _shard_inputs(
        arrs["buffer_states"], arrs["episodes_encoded"],
        arrs["W1"], arrs["b1"], arrs["g1"], arrs["be1"],
        arrs["W2"], arrs["b2"], arrs["g2"], arrs["be2"],
        arrs["W3"], arrs["b3"], arrs["Wd1"], arrs["bd1"], arrs["gd"],
        arrs["bed"], arrs["Wd2"], arrs["bd2"], zero_bias, unit_affine,
    )
    key = (zero_bias, unit_affine)
    if key not in _compiled:
        _compiled[key] = build_kernel(zero_bias=zero_bias,
                                      unit_affine=unit_affine)
    res = run_bass_kernel_spmd(_compiled[key], in_maps,
                               core_ids=list(range(NCORES)), trace=trace)
    out = _merge(res.results)
    if trace:
        kernel.last_exec_time_ns = res.exec_time_ns
        kernel.last_result = res
    return out


kernel.last_exec_time_ns = None


# revision 20
# speedup vs baseline: 1.1916x; 1.1916x over previous
"""EpisodicMemory retrieval kernel for 8 Trainium2 NeuronCores (v4).

Sharding (hardcoded for the nn_EpisodicMemory problem):
  - q = buffer_states.reshape(-1) [25600]: contraction-sharded for layer 1
    (core i gets q[3200i:3200(i+1)] and W1 rows [3200i:3200(i+1), :]),
    partial pre-activations summed with an on-device AllReduce (the only
    collective).
  - W2/W3 replicated; every core computes the full enc locally.
  - episodes_encoded row-sharded: core i scores episodes [1250i:1250(i+1))
    against enc entirely on the TensorEngine: the host supplies the shard
    TRANSPOSED ([128, 32, 1250] fp8), dots = enc-column x epT matmuls and
    norms = ones x square(epT) matmuls, both accumulating in flat [1,1250]
    PSUM (no layout bounces). Local top-8 via max8/max_index, exact fp32
    rescore of the 8 (indirect gather, 128KB), decode all 8 with bf16
    Wd1/Wd2.
  - host merges the 8x8 candidates: global top-3 by exact sims, then
    means the matching decoded vectors (pure gather/selection glue).

All bulk tensors are pre-swizzled on the host into their exact SBUF
layout ([128 partitions, ...] C-order) so each stream is a handful of
big contiguous DMAs (16-engine, ~25KB per partition line) instead of
thousands of 1KB descriptors.

Precision (validated in fp64 numpy against this dataset): candidate
generation runs fully in fp8 (weights pre-scaled by 64 to clear the fp8
subnormal range; enc scale is ranking-invariant), candidate selection
uses exact fp32 rows vs fp32 enc (margin 1.3e-3 vs noise <1e-4), decode
uses fp32 rows + bf16 weights (~4e-3 output rel err).
"""

import numpy as np

DIM = 256
WIN = 100
COMP = 16
NEP = 10000
NCORES = 8

Q = WIN * DIM            # 25600
H1 = 4 * DIM             # 1024
H2 = 2 * DIM             # 512
E = COMP * DIM           # 4096
QS = Q // NCORES         # 3200 rows of W1 per core
ES = NEP // NCORES       # 1250 episodes per core
K = 3
NC = 8                   # candidates per core (top-8 window)
EPS = 1e-5
W_SCALE = 64.0           # host multiplies W1/W2/W3 by this before fp8 cast

_compiled = {}


def build_kernel(gelu_func_name: str = "Gelu", zero_bias=False, unit_affine=False):
    import concourse.bacc as bacc
    import concourse.bass as bass
    import concourse.tile as tile
    import concourse.mybir as mybir

    f32 = mybir.dt.float32
    u32 = mybir.dt.uint32
    bf16 = mybir.dt.bfloat16
    fp8 = mybir.dt.float8e4
    AF = mybir.ActivationFunctionType
    GELU = getattr(AF, gelu_func_name)
    OP = mybir.AluOpType
    DS = 1.0 / W_SCALE

    nc = bacc.Bacc("TRN2", target_bir_lowering=False, debug=False,
                   enable_asserts=True, num_devices=NCORES)

    C1 = H1 // 128   # 8
    C2 = H2 // 128   # 4
    NKC = QS // 128  # 25
    CE = E // 128    # 32
    # episode column chunks for the 1250-wide PE accumulations (<=512 each)
    NSPLIT = [(0, 512), (512, 1024), (1024, ES)]

    # ---- I/O (all bulk tensors pre-swizzled to SBUF layout on host) ----
    q_s = nc.dram_tensor("q_s", [128, NKC], fp8, kind="ExternalInput").ap()
    W1sw = nc.dram_tensor("W1sw", [128, NKC, H1], fp8, kind="ExternalInput").ap()
    W2sw = nc.dram_tensor("W2sw", [128, C1, H2], fp8, kind="ExternalInput").ap()
    W3sw = nc.dram_tensor("W3sw", [128, C2, E], fp8, kind="ExternalInput").ap()
    epT = nc.dram_tensor("epT", [128, CE, ES], fp8, kind="ExternalInput").ap()
    ep32 = nc.dram_tensor("ep32", [ES, E], f32, kind="ExternalInput").ap()
    Wd1sw = nc.dram_tensor("Wd1sw", [128, CE, H2], bf16, kind="ExternalInput").ap()
    Wd2sw = nc.dram_tensor("Wd2sw", [128, C2, DIM], bf16, kind="ExternalInput").ap()
    eye8 = nc.dram_tensor("eye8", [NC, NC], f32, kind="ExternalInput").ap()
    vecs = {}
    if not zero_bias:
        for nm, width in [("b1v", H1), ("b2v", H2), ("b3v", E), ("bd1v", H2),
                          ("bd2v", DIM)]:
            vecs[nm] = nc.dram_tensor(nm, [width], f32, kind="ExternalInput").ap()
    if not unit_affine:
        for nm, width in [("g1v", H1), ("be1v", H1), ("g2v", H2), ("be2v", H2),
                          ("gdv", H2), ("bedv", H2)]:
            vecs[nm] = nc.dram_tensor(nm, [width], f32, kind="ExternalInput").ap()

    loc_out = nc.dram_tensor("loc_out", [NC, DIM], f32, kind="ExternalOutput").ap()
    loc_sims = nc.dram_tensor("loc_sims", [NC], f32, kind="ExternalOutput").ap()

    with tile.TileContext(nc) as tc:
        with tc.tile_pool(name="dram", bufs=1, space="DRAM") as dram, \
             tc.tile_pool(name="const", bufs=1) as const, \
             tc.tile_pool(name="small", bufs=1) as small, \
             tc.tile_pool(name="sqp", bufs=3) as sqp, \
             tc.tile_pool(name="psum", bufs=2, space="PSUM") as psum, \
             tc.tile_pool(name="psum_flat", bufs=1, space="PSUM") as psum_flat, \
             tc.tile_pool(name="psum_tp", bufs=2, space="PSUM") as psum_tp:

            # ---------- bulk streams on the sync ring, priority order ----------
            qsb = const.tile([128, NKC], fp8, tag="qsb")
            nc.sync.dma_start(out=qsb[:, :], in_=q_s[:, :])
            w1sb = const.tile([128, NKC, H1], fp8, tag="w1sb")
            bounds1 = [0, 7, 13, 19, NKC]
            for c in range(4):
                a, b = bounds1[c], bounds1[c + 1]
                nc.sync.dma_start(out=w1sb[:, a:b, :], in_=W1sw[:, a:b, :])

            epsb = const.tile([128, CE, ES], fp8, tag="epsb")
            for c in range(4):
                nc.sync.dma_start(out=epsb[:, 8 * c:8 * (c + 1), :],
                                  in_=epT[:, 8 * c:8 * (c + 1), :])

            w2sb = const.tile([128, C1, H2], fp8, tag="w2sb")
            nc.sync.dma_start(out=w2sb[:, :, :], in_=W2sw[:, :, :])
            w3sb = const.tile([128, C2, E], fp8, tag="w3sb")
            nc.sync.dma_start(out=w3sb[:, :, :], in_=W3sw[:, :, :])
            wd1sb = const.tile([128, CE, H2], bf16, tag="wd1sb")
            for c in range(2):
                nc.sync.dma_start(out=wd1sb[:, 16 * c:16 * (c + 1), :],
                                  in_=Wd1sw[:, 16 * c:16 * (c + 1), :])
            wd2sb = const.tile([128, C2, DIM], bf16, tag="wd2sb")
            nc.sync.dma_start(out=wd2sb[:, :, :], in_=Wd2sw[:, :, :])

            # ---------- small constants on the scalar ring ----------
            eye8sb = const.tile([NC, NC], f32, tag="eye8sb")
            nc.scalar.dma_start(out=eye8sb[:, :], in_=eye8[:, :])
            eye1 = const.tile([1, 1], f32, tag="eye1")
            nc.vector.memset(eye1[:, :], 1.0)
            ones_col = const.tile([128, 1], fp8, tag="ones_col")
            nc.vector.memset(ones_col[:, :], 1.0)
            eps1 = const.tile([1, 1], f32, tag="eps1")
            nc.vector.memset(eps1[:, :], EPS)
            eps8 = const.tile([NC, 1], f32, tag="eps8")
            nc.vector.memset(eps8[:, :], EPS)

            def cvec(nm, width, tag):
                t = const.tile([1, width], f32, tag=tag)
                nc.scalar.dma_start(
                    out=t[:, :], in_=vecs[nm].rearrange("(a n) -> a n", a=1))
                return t

            def cvec_b(nm, width, tag):
                t = const.tile([NC, width], f32, tag=tag)
                nc.scalar.dma_start(
                    out=t[:, :],
                    in_=vecs[nm].rearrange("(a n) -> a n", a=1)
                        .to_broadcast([NC, width]))
                return t

            b1sb = cvec("b1v", H1, "b1sb") if not zero_bias else None
            b2sb = cvec("b2v", H2, "b2sb") if not zero_bias else None
            b3sb = cvec("b3v", E, "b3sb") if not zero_bias else None
            bd1sb = cvec_b("bd1v", H2, "bd1sb") if not zero_bias else None
            bd2sb = cvec_b("bd2v", DIM, "bd2sb") if not zero_bias else None
            g1sb = cvec("g1v", H1, "g1sb") if not unit_affine else None
            be1sb = cvec("be1v", H1, "be1sb") if not unit_affine else None
            g2sb = cvec("g2v", H2, "g2sb") if not unit_affine else None
            be2sb = cvec("be2v", H2, "be2sb") if not unit_affine else None
            gdsb = cvec_b("gdv", H2, "gdsb") if not unit_affine else None
            bedsb = cvec_b("bedv", H2, "bedsb") if not unit_affine else None

            # DRAM bounce/scratch
            ar1_in = dram.tile([H1], f32)
            ar1_out = dram.tile([H1], f32)
            enc_dbf = dram.tile([E], bf16)
            enc_df = dram.tile([E], f32)
            idx_d = dram.tile([NC], u32)

            # ======== E1: h1_pre = q_s @ (64*W1) -> psum [1, 1024] ========
            e1pa = psum.tile([1, 512], f32, tag="mm")
            e1pb = psum.tile([1, 512], f32, tag="mm")
            for kc in range(NKC):
                for h, pp in ((0, e1pa), (1, e1pb)):
                    nc.tensor.matmul(
                        out=pp[:, :],
                        lhsT=qsb[:, kc:kc + 1],
                        rhs=w1sb[:, kc, 512 * h:512 * (h + 1)],
                        start=(kc == 0), stop=(kc == NKC - 1),
                    )
            h1f = small.tile([1, H1], f32, tag="h1flat")
            nc.vector.tensor_copy(out=h1f[:, :512], in_=e1pa[:, :])
            nc.vector.tensor_copy(out=h1f[:, 512:], in_=e1pb[:, :])
            nc.scalar.dma_start(out=ar1_in.rearrange("(a n) -> a n", a=1),
                                in_=h1f[:, :])
            nc.gpsimd.collective_compute(
                "AllReduce", OP.add,
                replica_groups=[list(range(NCORES))],
                ins=[ar1_in.opt()], outs=[ar1_out.opt()],
            )

            # ======== episode norms on PE (pre-enc, fills the AR window) ======
            # nsq[n] = sum_k epT[k,n]^2 = ones.T @ square(epT)
            nsq_p = psum_flat.tile([1, ES], f32, tag="flatp")
            for kc in range(CE):
                sq = sqp.tile([128, ES], fp8, tag="sq")
                nc.scalar.activation(out=sq[:, :], in_=epsb[:, kc, :],
                                     func=AF.Square)
                for ci, (a, b) in enumerate(NSPLIT):
                    nc.tensor.matmul(
                        out=nsq_p[:, a:b], lhsT=ones_col[:, :], rhs=sq[:, a:b],
                        start=(kc == 0), stop=(kc == CE - 1),
                    )
            nsqf = small.tile([1, ES], f32, tag="nsqf")
            nc.vector.tensor_copy(out=nsqf[:, :], in_=nsq_p[:, :])
            rstd = small.tile([1, ES], f32, tag="rstdf")
            nc.scalar.activation(out=rstd[:, :], in_=nsqf[:, :], func=AF.Sqrt)
            nc.vector.reciprocal(out=rstd[:, :], in_=rstd[:, :])

            def ln_flat(xf, xout, width, bsb, gsb, besb, name, pre_scale=1.0):
                """LN(gelu(xf*pre_scale + b)) on [1,width] f32 -> xout (f32)."""
                if bsb is not None:
                    if pre_scale != 1.0:
                        nc.scalar.activation(out=xf[:, :], in_=xf[:, :],
                                             func=AF.Copy, scale=pre_scale)
                    nc.vector.tensor_add(out=xf[:, :], in0=xf[:, :], in1=bsb[:, :])
                    nc.scalar.activation(out=xf[:, :], in_=xf[:, :], func=GELU)
                else:
                    nc.scalar.activation(out=xf[:, :], in_=xf[:, :], func=GELU,
                                         scale=pre_scale)
                nsub = (width + 511) // 512
                st = small.tile([1, nsub, 6], f32, tag=f"st_{name}")
                for sg in range(nsub):
                    nc.vector.bn_stats(out=st[:, sg, :],
                                       in_=xf[:, 512 * sg:512 * (sg + 1)])
                mv = small.tile([1, 2], f32, tag=f"mv_{name}")
                nc.vector.bn_aggr(out=mv[:, :], in_=st[:, :, :])
                rs = small.tile([1, 1], f32, tag=f"rstd_{name}")
                nc.scalar.activation(out=rs[:, :], in_=mv[:, 1:2], func=AF.Sqrt,
                                     bias=eps1[:, :])
                nc.vector.reciprocal(out=rs[:, :], in_=rs[:, :])
                last = xout if gsb is None else xf
                nc.vector.tensor_scalar(
                    out=last[:, :], in0=xf[:, :],
                    scalar1=mv[:, 0:1], scalar2=rs[:, :],
                    op0=OP.subtract, op1=OP.mult,
                )
                if gsb is not None:
                    nc.vector.tensor_mul(out=xf[:, :], in0=xf[:, :], in1=gsb[:, :])
                    nc.vector.tensor_add(out=xout[:, :], in0=xf[:, :], in1=besb[:, :])

            def col_pack(src, n_kc, dst):
                """src [1, n_kc*128] f32 -> dst [128, n_kc] (cast to dst dtype)
                via PE transposes (no DRAM bounce)."""
                for kc in range(n_kc):
                    tp = psum_tp.tile([128, NC], f32, tag="tp")
                    nc.tensor.transpose(out=tp[:, :1],
                                        in_=src[:, 128 * kc:128 * (kc + 1)],
                                        identity=eye1[:, :])
                    nc.vector.tensor_copy(out=dst[:, kc:kc + 1], in_=tp[:, :1])

            # ---------- E1 epilogue ----------
            h1 = small.tile([1, H1], f32, tag="h1flat")
            nc.scalar.dma_start(out=h1[:, :],
                                in_=ar1_out.rearrange("(a n) -> a n", a=1))
            h1n = small.tile([1, H1], f32, tag="h1n")
            ln_flat(h1, h1n, H1, b1sb, g1sb, be1sb, "l1", pre_scale=DS)
            h1m = small.tile([128, C1], fp8, tag="h1m")
            col_pack(h1n, C1, h1m)

            # ======== E2 ========
            e23p = psum.tile([1, H2], f32, tag="mm")
            for kc in range(C1):
                nc.tensor.matmul(
                    out=e23p[:, :], lhsT=h1m[:, kc:kc + 1], rhs=w2sb[:, kc, :],
                    start=(kc == 0), stop=(kc == C1 - 1),
                )
            h2 = small.tile([1, H2], f32, tag="h2flat")
            nc.vector.tensor_copy(out=h2[:, :], in_=e23p[:, :])
            h2n = small.tile([1, H2], f32, tag="h2n")
            ln_flat(h2, h2n, H2, b2sb, g2sb, be2sb, "l2", pre_scale=DS)
            h2m = small.tile([128, C2], fp8, tag="h2m")
            col_pack(h2n, C2, h2m)

            # ======== E3: full enc = h2 @ W3 (descale on PSUM copy) ========
            encf = small.tile([1, E], f32, tag="encf")
            for cg in range(4):
                for h in range(2):
                    e3p = psum.tile([1, 512], f32, tag="mm")
                    for kc in range(C2):
                        nc.tensor.matmul(
                            out=e3p[:, :],
                            lhsT=h2m[:, kc:kc + 1],
                            rhs=w3sb[:, kc,
                                     1024 * cg + 512 * h:1024 * cg + 512 * (h + 1)],
                            start=(kc == 0), stop=(kc == C2 - 1),
                        )
                    nc.vector.tensor_scalar_mul(
                        out=encf[:, 1024 * cg + 512 * h:1024 * cg + 512 * (h + 1)],
                        in0=e3p[:, :], scalar1=DS)
            if b3sb is not None:
                nc.vector.tensor_add(out=encf[:, :], in0=encf[:, :], in1=b3sb[:, :])

            # enc -> [128, 32] fp8 columns via DMA-transpose (bf16 bounce)
            encbf = small.tile([1, E], bf16, tag="encbf")
            nc.vector.tensor_copy(out=encbf[:, :], in_=encf[:, :])
            nc.scalar.dma_start(out=enc_dbf.rearrange("(a n) -> a n", a=1),
                                in_=encbf[:, :])
            encm_bf = small.tile([128, CE], bf16, tag="encm_bf")
            nc.sync.dma_start_transpose(
                out=encm_bf[:, :], in_=enc_dbf.rearrange("(kc p) -> kc p", p=128))
            encm = small.tile([128, CE], fp8, tag="encm")
            nc.vector.tensor_copy(out=encm[:, :], in_=encm_bf[:, :])
            # enc broadcast to 8 partitions (f32) for the exact rescore
            nc.scalar.dma_start(out=enc_df.rearrange("(a n) -> a n", a=1),
                                in_=encf[:, :])
            enc8b = small.tile([NC, E], f32, tag="enc8b")
            nc.scalar.dma_start(
                out=enc8b[:, :],
                in_=enc_df.rearrange("(a n) -> a n", a=1).to_broadcast([NC, E]))

            # ======== dots on PE: dot[n] = sum_kc enc_col(kc) . epT[kc][:,n] ====
            dot_p = psum_flat.tile([1, ES], f32, tag="flatp")
            for kc in range(CE):
                for a, b in NSPLIT:
                    nc.tensor.matmul(
                        out=dot_p[:, a:b], lhsT=encm[:, kc:kc + 1],
                        rhs=epsb[:, kc, a:b],
                        start=(kc == 0), stop=(kc == CE - 1),
                    )
            # ======== normalize + local top-8 (read dots from PSUM) ========
            snorm = small.tile([1, ES], f32, tag="snorm")
            nc.vector.tensor_mul(out=snorm[:, :], in0=dot_p[:, :], in1=rstd[:, :])
            vals = small.tile([1, NC], f32, tag="vals")
            nc.vector.max(out=vals[:, :], in_=snorm[:, :])
            idx8 = small.tile([1, NC], u32, tag="idx8")
            nc.vector.max_index(out=idx8[:, :], in_max=vals[:, :],
                                in_values=snorm[:, :])
            nc.scalar.dma_start(out=idx_d.rearrange("(a n) -> a n", a=1),
                                in_=idx8[:, :])
            idxc = small.tile([NC, 1], u32, tag="idxc")
            nc.scalar.dma_start(out=idxc[:, :],
                                in_=idx_d.rearrange("(p o) -> p o", o=1))

            # ======== exact rescore of the 8 candidates ========
            rows8 = small.tile([NC, E], f32, tag="encf")  # reuses encf
            nc.gpsimd.indirect_dma_start(
                out=rows8[:, :], out_offset=None,
                in_=ep32[:, :],
                in_offset=bass.IndirectOffsetOnAxis(ap=idxc[:, :1], axis=0),
            )
            trash8 = small.tile([NC, E], bf16, tag="trash8")
            dots8 = small.tile([NC, 1], f32, tag="dots8")
            nsq8 = small.tile([NC, 1], f32, tag="nsq8")
            nc.vector.tensor_tensor(out=trash8[:, :], in0=rows8[:, :],
                                    in1=enc8b[:, :], op=OP.mult)
            nc.vector.tensor_reduce(out=dots8[:, :], in_=trash8[:, :],
                                    axis=mybir.AxisListType.X, op=OP.add)
            nc.scalar.activation(out=trash8[:, :], in_=rows8[:, :],
                                 func=AF.Square, accum_out=nsq8[:, :])
            nstd8 = small.tile([NC, 1], f32, tag="nstd8")
            nc.scalar.activation(out=nstd8[:, :], in_=nsq8[:, :], func=AF.Sqrt)
            nc.vector.reciprocal(out=nstd8[:, :], in_=nstd8[:, :])
            sim8 = small.tile([NC, 1], f32, tag="sim8")
            nc.vector.tensor_mul(out=sim8[:, :], in0=dots8[:, :], in1=nstd8[:, :])
            nc.scalar.dma_start(out=loc_sims.rearrange("(p o) -> p o", o=1),
                                in_=sim8[:, :])

            # ======== decoder: all 8 candidates ========
            rowsT = small.tile([128, CE, NC], bf16, tag="rowsT")
            pdp = psum.tile([NC, H2], f32, tag="mm")
            for kc in range(CE):
                tp = psum_tp.tile([128, NC], f32, tag="tp")
                nc.tensor.transpose(out=tp[:, :],
                                    in_=rows8[:, 128 * kc:128 * (kc + 1)],
                                    identity=eye8sb[:, :])
                nc.vector.tensor_copy(out=rowsT[:, kc, :], in_=tp[:, :])
            for kc in range(CE):
                nc.tensor.matmul(
                    out=pdp[:, :], lhsT=rowsT[:, kc, :], rhs=wd1sb[:, kc, :],
                    start=(kc == 0), stop=(kc == CE - 1),
                )
            d = small.tile([NC, H2], f32, tag="d")
            nc.vector.tensor_copy(out=d[:, :], in_=pdp[:, :])
            if bd1sb is not None:
                nc.vector.tensor_add(out=d[:, :], in0=d[:, :], in1=bd1sb[:, :])
            nc.scalar.activation(out=d[:, :], in_=d[:, :], func=GELU)
            std = small.tile([NC, 6], f32, tag="std")
            nc.vector.bn_stats(out=std[:, :], in_=d[:, :])
            mvd = small.tile([NC, 2], f32, tag="mvd")
            nc.vector.bn_aggr(out=mvd[:, :], in_=std[:, :])
            rstdd = small.tile([NC, 1], f32, tag="rstdd")
            nc.scalar.activation(out=rstdd[:, :], in_=mvd[:, 1:2], func=AF.Sqrt,
                                 bias=eps8[:, :])
            nc.vector.reciprocal(out=rstdd[:, :], in_=rstdd[:, :])
            nc.vector.tensor_scalar(
                out=d[:, :], in0=d[:, :],
                scalar1=mvd[:, 0:1], scalar2=rstdd[:, :],
                op0=OP.subtract, op1=OP.mult,
            )
            if gdsb is not None:
                nc.vector.tensor_mul(out=d[:, :], in0=d[:, :], in1=gdsb[:, :])
                nc.vector.tensor_add(out=d[:, :], in0=d[:, :], in1=bedsb[:, :])

            dT = small.tile([128, C2, NC], bf16, tag="dT")
            o3p = psum.tile([NC, DIM], f32, tag="mm")
            for kc in range(C2):
                tp = psum_tp.tile([128, NC], f32, tag="tp")
                nc.tensor.transpose(out=tp[:, :],
                                    in_=d[:, 128 * kc:128 * (kc + 1)],
                                    identity=eye8sb[:, :])
                nc.vector.tensor_copy(out=dT[:, kc, :], in_=tp[:, :])
                nc.tensor.matmul(
                    out=o3p[:, :], lhsT=dT[:, kc, :], rhs=wd2sb[:, kc, :],
                    start=(kc == 0), stop=(kc == C2 - 1),
                )
            o3 = small.tile([NC, DIM], f32, tag="o3")
            nc.vector.tensor_copy(out=o3[:, :], in_=o3p[:, :])
            if bd2sb is not None:
                nc.vector.tensor_add(out=o3[:, :], in0=o3[:, :], in1=bd2sb[:, :])

            nc.sync.dma_start(out=loc_out[:, :], in_=o3[:, :])

    nc.compile()
    return nc


def _bf16(a):
    import ml_dtypes
    return np.ascontiguousarray(
        np.asarray(a, dtype=np.float32).astype(ml_dtypes.bfloat16))


def _fp8(a):
    import ml_dtypes
    return np.ascontiguousarray(
        np.asarray(a, dtype=np.float32).astype(ml_dtypes.float8_e4m3))


def _swizzle(w, n_kc):
    """[n_kc*128, n] row-major -> [128, n_kc, n] C-order (SBUF layout)."""
    w = np.asarray(w, dtype=np.float32)
    n = w.shape[1]
    return np.ascontiguousarray(
        w.reshape(n_kc, 128, n).transpose(1, 0, 2))


def _shard_inputs(buffer_states, episodes_encoded, W1, b1, g1, be1, W2, b2, g2,
                  be2, W3, b3, Wd1, bd1, gd, bed, Wd2, bd2, zero_bias,
                  unit_affine):
    q = np.ascontiguousarray(buffer_states, dtype=np.float32).reshape(-1)
    eye8 = np.eye(NC, dtype=np.float32)
    W2c = _fp8(_swizzle(np.asarray(W2, dtype=np.float32) * W_SCALE, H1 // 128))
    W3c = _fp8(_swizzle(np.asarray(W3, dtype=np.float32) * W_SCALE, H2 // 128))
    Wd1c = _bf16(_swizzle(Wd1, E // 128))
    Wd2c = _bf16(_swizzle(Wd2, H2 // 128))
    ep32 = np.ascontiguousarray(episodes_encoded, dtype=np.float32)
    in_maps = []
    for i in range(NCORES):
        qs = q[QS * i:QS * (i + 1)]
        shard = ep32[ES * i:ES * (i + 1)]                     # [1250, 4096]
        # epT [128, 32, 1250]: epT[p, kc, n] = shard[n, 128*kc + p]
        epTc = _fp8(np.ascontiguousarray(
            shard.T.reshape(E // 128, 128, ES).transpose(1, 0, 2)))
        m = {
            "q_s": _fp8(np.ascontiguousarray(qs.reshape(QS // 128, 128).T)),
            "W1sw": _fp8(_swizzle(
                np.asarray(W1[QS * i:QS * (i + 1)], dtype=np.float32) * W_SCALE,
                QS // 128)),
            "W2sw": W2c,
            "W3sw": W3c,
            "epT": epTc,
            "ep32": shard,
            "Wd1sw": Wd1c,
            "Wd2sw": Wd2c,
            "eye8": eye8,
        }
        if not zero_bias:
            m.update({"b1v": b1, "b2v": b2, "b3v": b3, "bd1v": bd1, "bd2v": bd2})
        if not unit_affine:
            m.update({"g1v": g1, "be1v": be1, "g2v": g2, "be2v": be2,
                      "gdv": gd, "bedv": bed})
        in_maps.append(m)
    return in_maps


def _merge(results):
    sims = np.concatenate([r["loc_sims"] for r in results])              # [64]
    outs = np.concatenate([r["loc_out"] for r in results], axis=0)       # [64, 256]
    top = np.argsort(-sims, kind="stable")[:K]
    return outs[top].mean(axis=0).astype(np.float32)


def kernel(*, trace=False, **inputs):
    from concourse.bass_utils import run_bass_kernel_spmd

    k = int(inputs.pop("k"))
    assert k == K, f"kernel hardcodes k=3, got {k}"
    arrs = {name: np.ascontiguousarray(np.asarray(v, dtype=np.float32))
            for name, v in inputs.items()}
    zero_bias = all(not arrs[n].any() for n in ("b1", "b2", "b3", "bd1", "bd2"))
    unit_affine = (all(np.all(arrs[n] == 1.0) for n in ("g1", "g2", "gd")) and
                   all(not arrs[n].any() for n in ("be1", "be2", "bed")))
    in_maps = _shard_inputs(
        arrs["buffer_states"], arrs["episodes_encoded"],
        arrs["W1"], arrs["b1"], arrs["g1"], arrs["be1"],
        arrs["W2"], arrs["b2"], arrs["g2"], arrs["be2"],
        arrs["W3"], arrs["b3"], arrs["Wd1"], arrs["bd1"], arrs["gd"],
        arrs["bed"], arrs["Wd2"], arrs["bd2"], zero_bias, unit_affine,
    )
    key = (zero_bias, unit_affine)
    if key not in _compiled:
        _compiled[key] = build_kernel(zero_bias=zero_bias,
                                      unit_affine=unit_affine)
    res = run_bass_kernel_spmd(_compiled[key], in_maps,
                               core_ids=list(range(NCORES)), trace=trace)
    out = _merge(res.results)
    if trace:
        kernel.last_exec_time_ns = res.exec_time_ns
        kernel.last_result = res
    return out


kernel.last_exec_time_ns = None
